# revision 50
# baseline (speedup 1.0000x reference)
"""Trainium2 Bass kernel for a 2-layer GATv2 + global mean pool (GNN message passing).

Strategy (8 NeuronCores, SPMD):
  - Host: sort edges by target node; partition nodes into 8 equal contiguous
    ranges; each core owns all in-edges of its node range, so softmax
    segment-reductions are fully core-local.
  - Edges are grouped into 128-node windows; each window's edge list is padded
    to a fixed number of 128-edge subtiles (T) so the kernel is static.
  - Layer 1: input features are tiny (2-d), so per-edge features are shipped as
    a 7-row "cat" matrix ([x_src; x_tgt; edge_attr; 1]) and all node/edge
    transforms become small matmuls.  |att| is folded into the weights so the
    logit head-reduction is a matmul too (signs in an R matrix); segment-sums
    are one-hot matmuls accumulated in PSUM (edge-sorted => local windows).
  - Between layers: AllGather of the per-node xl2 table (h @ W_l2), staged
    in 3 skewed chunks (20/40/40% of nodes) so layer-2 gathers start early.
  - Layer 1 values use a rank-3 trick: the weighted-value segment-sum only
    accumulates (ee*x_src, ee) 3-vectors per head; node features emerge via
    a block-diagonal [48,128] matmul after the softmax division.
  - Layer 2: xl2[src] rows fetched with dma_gather (int16 indices, 4 SWDGE
    queues round-robin to parallelize Q7 descriptor generation);
    xr2[tgt] + xe2 come from ONE one-hot matmul per subtile (windows hold
    126 nodes; one-hot rows 126-127 carry edge attrs, xr2 rows carry We2);
    xg enters the same PSUM group via an identity matmul.  Same one-hot
    segment-sum, then a one-hot matmul vs graph ids for global pooling.
    Host sums the 8 per-core pooled partials and divides by graph sizes.
"""
import math
import numpy as np
import ml_dtypes

import concourse.bass as bass
import concourse.tile as tile
from concourse import bacc, mybir
from concourse.bass_utils import run_bass_kernel_spmd

F32 = mybir.dt.float32
F32R = mybir.dt.float32r
BF16 = mybir.dt.bfloat16
I32 = mybir.dt.int32
I16 = mybir.dt.int16
BF16NP = ml_dtypes.bfloat16

N, E, G = 50000, 800000, 64
NCORES = 8
NLOC = N // NCORES            # 6250 nodes per core
WINP = 128                    # slots per subtile / partitions
WN = 126                      # nodes per window (rows 126-127 carry We2)
NW = (NLOC + WN - 1) // WN    # 50 windows
H1, C1, D1 = 16, 8, 128
H2, C2, D2 = 4, 16, 64
LRELU_ALPHA = 0.2
PAD_SENTINEL = -5.0
STAGE_FR = (0.0, 0.2, 0.6, 1.0)   # skewed stage bounds: small stage 0


# --------------------------------------------------------------------------
# Host-side preprocessing
# --------------------------------------------------------------------------
def _prep(inputs):
    x = np.ascontiguousarray(np.asarray(inputs["x"], dtype=np.float32))
    ea = np.ascontiguousarray(np.asarray(inputs["edge_attr"], dtype=np.float32))
    ei = np.asarray(inputs["edge_index"])
    batch = np.asarray(inputs["batch"]).astype(np.int64)
    src = ei[0].astype(np.int64)
    tgt = ei[1].astype(np.int64)

    order = np.argsort(tgt, kind="stable")
    src_s = src[order]
    tgt_s = tgt[order]
    ea_s = ea[order]

    # per-(core,window,stage) edge counts -> Tk (subtiles per window stage)
    NST = 3
    bounds = [round(NLOC * STAGE_FR[k]) for k in range(NST + 1)]
    qsize = [bounds[k + 1] - bounds[k] for k in range(NST)]
    Tk = [1] * NST
    seg = []
    for d in range(NCORES):
        lo, hi = np.searchsorted(tgt_s, [d * NLOC, (d + 1) * NLOC])
        ltgt = tgt_s[lo:hi] - d * NLOC
        stage = np.searchsorted(bounds, src_s[lo:hi] % NLOC, "right") - 1
        w = ltgt // WN
        for k in range(NST):
            ck = np.bincount(w[stage == k], minlength=NW)
            Tk[k] = max(Tk[k], int(math.ceil(ck.max() / WINP)))
        seg.append((lo, hi))
    prefT = np.concatenate([[0], np.cumsum(Tk)]).astype(np.int64)
    T = int(prefT[-1])
    EPW = T * WINP          # edge slots per window
    NSUB = NW * T           # subtiles per core
    EP = NW * EPW           # edge slots per core

    # constants (shared by all cores)
    f32 = np.float32
    W_l1 = np.asarray(inputs["W_l1"], f32); b_l1 = np.asarray(inputs["b_l1"], f32)
    W_r1 = np.asarray(inputs["W_r1"], f32); b_r1 = np.asarray(inputs["b_r1"], f32)
    W_e1 = np.asarray(inputs["W_e1"], f32)
    att1 = np.asarray(inputs["att1"], f32)
    bias1 = np.asarray(inputs["bias1"], f32)
    W_l2 = np.asarray(inputs["W_l2"], f32); b_l2 = np.asarray(inputs["b_l2"], f32)
    W_r2 = np.asarray(inputs["W_r2"], f32); b_r2 = np.asarray(inputs["b_r2"], f32)
    W_e2 = np.asarray(inputs["W_e2"], f32)
    att2 = np.asarray(inputs["att2"], f32)
    bias2 = np.asarray(inputs["bias2"], f32)

    att1f = att1.reshape(D1)
    wcats = np.concatenate(
        [W_l1, W_r1, W_e1, (b_l1 + b_r1)[None, :]], axis=0
    ) * np.abs(att1f)[None, :]                                   # [7,128]
    rsign = np.zeros((D1, H1), f32)
    rsign[np.arange(D1), np.arange(D1) // C1] = np.sign(att1f)    # [128,16]
    # block-diagonal [48,128]: row (h,k) carries W_l1[k] (k=0,1) / b_l1 (k=2)
    # restricted to head h's feature block
    bd48 = np.zeros((3 * H1, D1), f32)
    for h in range(H1):
        s8 = slice(h * C1, (h + 1) * C1)
        bd48[h * 3 + 0, s8] = W_l1[0, s8]
        bd48[h * 3 + 1, s8] = W_l1[1, s8]
        bd48[h * 3 + 2, s8] = b_l1[s8]
    w2cat = np.concatenate([W_l2, W_r2], axis=1)                  # [128,128]
    b2cor = np.concatenate([b_l2, b_r2]) - w2cat.sum(axis=0)
    b2rep = np.tile(b2cor[None, :], (WINP, 1))
    att2gb = np.tile(att2.reshape(D2)[None, :], (WINP, 8))        # [128,8*64]
    bias1rep = np.tile(bias1[None, :], (WINP, 1))                 # [128,128]
    bias2rep = np.tile(bias2[None, :], (WINP, 1))                 # [128,64]
    iota64 = np.tile(np.arange(G, dtype=f32)[None, :], (WINP, 1))
    ident = np.eye(WINP, dtype=f32)

    xr2init = np.zeros((NW * WINP, D2), BF16NP)
    xr2init.reshape(NW, WINP, D2)[:, WN:WINP, :] = W_e2.astype(BF16NP)
    consts = dict(
        wcats=wcats.astype(BF16NP), rsign=rsign.astype(BF16NP), bd48=bd48,
        xr2loc=xr2init,
        w2cat=w2cat.astype(f32), b2rep=b2rep.astype(f32),
        att2gb=att2gb.astype(BF16NP), bias1rep=bias1rep.astype(f32),
        bias2rep=bias2rep.astype(f32), iota64=iota64,
        ident=ident,
    )

    in_maps = []
    for d in range(NCORES):
        lo, hi = seg[d]
        ssrc = src_s[lo:hi]
        ltgt = tgt_s[lo:hi] - d * NLOC
        sea = ea_s[lo:hi]
        w = ltgt // WN
        stage = np.searchsorted(bounds, ssrc % NLOC, "right") - 1
        # sort edges by (window, stage), stable
        order2 = np.lexsort((stage, w))
        ssrc = ssrc[order2]; ltgt = ltgt[order2]; sea = sea[order2]
        w = w[order2]; stage = stage[order2]
        # rank within (window, stage)
        key = w * NST + stage
        kstart = np.searchsorted(key, np.arange(NST * NW))
        rank = np.arange(len(w)) - kstart[key]
        g = w * EPW + prefT[stage] * WINP + rank                 # global slot
        # remapped gather index into the stage buffers
        sdev = ssrc // NLOC
        spos = ssrc % NLOC - np.asarray(bounds)[stage]
        gidx = sdev * np.asarray(qsize)[stage] + spos

        catf = np.zeros((7, EP), f32)
        catf[0:2, g] = x[ssrc].T
        catf[2:4, g] = x[ltgt + d * NLOC].T
        catf[4:6, g] = sea.T
        catf[6, g] = 1.0
        cat16 = np.ascontiguousarray(
            catf.reshape(7, NW, EPW).transpose(1, 0, 2)).astype(BF16NP)  # [NW,7,EPW]
        # slot-major source features: cat2s[p, su*2+d] = x[src of slot (su,p)][d]
        cat2s = np.ascontiguousarray(
            catf[0:2].reshape(2, NSUB, WINP).transpose(2, 1, 0)
        ).reshape(WINP, 2 * NSUB).astype(BF16NP)

        tsh = np.full(EP, PAD_SENTINEL, f32)
        tsh[g] = (ltgt - w * WN).astype(f32)
        tshift = np.ascontiguousarray(tsh.reshape(NSUB, WINP).T)  # [128,NSUB]

        sidx = np.zeros(EP, np.int32)
        sidx[g] = gidx.astype(np.int32)
        # dma_gather index layout, stage-major subtile order so a run of
        # windows within one stage is contiguous:
        # block index (k, w, su) -> prefT[k]*NW + w*Tk[k] + su
        s3 = sidx.reshape(NW, T, WINP)
        sidx2 = np.concatenate(
            [s3[:, prefT[k]:prefT[k] + Tk[k], :].reshape(NW * Tk[k], WINP)
             for k in range(NST)], axis=0)
        # wrapped: idx16[p%16, b*8+s] = sidx2[b, s*16 + p%16], replicated x8
        s16 = sidx2.reshape(NSUB, 8, 16).transpose(2, 0, 1).reshape(16, NSUB * 8)
        idx16 = np.ascontiguousarray(np.tile(s16, (8, 1))).astype(np.int16)

        tidx = np.zeros(EP, np.int32)
        tidx[g] = ltgt.astype(np.int32)
        tgtidx = np.ascontiguousarray(tidx.reshape(NSUB, WINP).T)

        # host-precomputed transposed one-hot: St[n, slot] = (tshift[slot] == n);
        # rows 126-127 carry the slot's edge_attr (pairs with We2 rows of xr2)
        stw = np.zeros((NW, WINP, EPW), BF16NP)
        tsh2 = tsh.reshape(NW, EPW)
        ea2 = catf[4:6].reshape(2, NW, EPW)
        for w in range(NW):
            valid = tsh2[w] >= 0
            cols = np.nonzero(valid)[0]
            stw[w, tsh2[w, cols].astype(int), cols] = 1.0
            stw[w, WN:WINP, :] = ea2[:, w, :].astype(BF16NP)
        # edge-major one-hot: S[p, su*128 + n] = (tshift[su,p] == n)
        sw = np.zeros((NW, WINP, EPW), BF16NP)
        tsh3 = tsh.reshape(NW, T, WINP)
        for w in range(NW):
            su_i, p_i = np.nonzero(tsh3[w] >= 0)
            n_i = tsh3[w, su_i, p_i].astype(int)
            sw[w, p_i, su_i * WINP + n_i] = 1.0

        batchw = np.full((WINP, NW), PAD_SENTINEL, f32)
        nodes = np.arange(NLOC)
        batchw[nodes % WN, nodes // WN] = batch[d * NLOC + nodes].astype(f32)

        m = dict(cat16=cat16, cat2s=cat2s, tshift=tshift, idx16=idx16,
                 stw=stw, sw=sw, batchw=batchw)
        m.update(consts)
        in_maps.append(m)

    counts = np.bincount(batch, minlength=G).astype(np.float32)
    return in_maps, tuple(Tk), counts


# --------------------------------------------------------------------------
# Device program
# --------------------------------------------------------------------------
DEBUG = False

def _build(*Tk):
    NST = len(Tk)
    T = sum(Tk)
    prefT = [0]
    for t in Tk:
        prefT.append(prefT[-1] + t)
    bounds = [round(NLOC * STAGE_FR[k]) for k in range(NST + 1)]
    qsize = [bounds[k + 1] - bounds[k] for k in range(NST)]
    EPW = T * WINP
    NSUB = NW * T
    GBMAX = 8
    wcut = [(bounds[k + 1] + WN - 1) // WN for k in range(NST)]

    nc = bacc.Bacc("TRN2", target_bir_lowering=False, debug=False,
                   num_devices=NCORES, enable_asserts=False,
                   num_swdge_queues=4)

    def din(name, shape, dt):
        return nc.dram_tensor(name, shape, dt, kind="ExternalInput").ap()

    cat16 = din("cat16", [NW, 7, EPW], BF16)
    cat2s = din("cat2s", [WINP, 2 * NSUB], BF16)
    idx16 = din("idx16", [WINP, NSUB * 8], I16)
    stw = din("stw", [NW, WINP, EPW], BF16)
    sw = din("sw", [NW, WINP, EPW], BF16)
    batchw = din("batchw", [WINP, NW], F32)
    wcats = din("wcats", [7, D1], BF16)
    rsign = din("rsign", [D1, H1], BF16)
    bd48 = din("bd48", [3 * H1, D1], F32)
    w2cat = din("w2cat", [D1, D1], F32)
    b2rep = din("b2rep", [WINP, D1], F32)
    att2gb = din("att2gb", [WINP, 8 * D2], BF16)
    bias1rep = din("bias1rep", [WINP, D1], F32)
    bias2rep = din("bias2rep", [WINP, D2], F32)
    iota64 = din("iota64", [WINP, G], F32)
    ident = din("ident", [WINP, WINP], F32)

    pool_out = nc.dram_tensor("pool_out", [G, D2], F32, kind="ExternalOutput").ap()
    if DEBUG:
        dbg_h = nc.dram_tensor("dbg_h", [NW * WINP, D1], F32, kind="ExternalOutput").ap()
        dbg_h2 = nc.dram_tensor("dbg_h2", [NW * WINP, D2], F32, kind="ExternalOutput").ap()
        dbg_den = nc.dram_tensor("dbg_den", [NW * WINP, H1], F32, kind="ExternalOutput").ap()
        dbg_m2 = nc.dram_tensor("dbg_m2", [WINP, 6 * D2], F32, kind="ExternalOutput").ap()
        dbg_xg = nc.dram_tensor("dbg_xg", [WINP, 6 * D2], F32, kind="ExternalOutput").ap()
        dbg_m2p = nc.dram_tensor("dbg_m2p", [WINP, 6 * D2], F32, kind="ExternalOutput").ap()

    xl2locs = [nc.dram_tensor("xl2loc%d" % k, [qsize[k], D2], F32).ap()
               for k in range(NST)]
    xr2loc = din("xr2loc", [NW * WINP, D2], BF16)
    xl2buf = [nc.dram_tensor("xl2buf%d" % k, [NCORES * qsize[k], D2], F32,
                              addr_space="Shared").ap() for k in range(NST)]

    AT = mybir.ActivationFunctionType
    with tile.TileContext(nc) as tc:
        with tc.tile_pool(name="const", bufs=1) as cpool, \
             tc.tile_pool(name="idx", bufs=1) as ipool, \
             tc.tile_pool(name="catw", bufs=4) as catpool, \
             tc.tile_pool(name="work", bufs=6) as wpool, \
             tc.tile_pool(name="acc", bufs=1, space="PSUM") as psA, \
             tc.tile_pool(name="big", bufs=3, space="PSUM") as psB:

            def cload(ap, shape, dt, tag):
                t = cpool.tile(shape, dt, tag=tag)
                nc.sync.dma_start(out=t[:], in_=ap[:, :])
                return t

            wcats_sb = cload(wcats, [7, D1], BF16, "k_wcats")
            rsign_sb = cload(rsign, [D1, H1], BF16, "k_rsign")
            bd48_sb = cload(bd48, [3 * H1, D1], F32, "k_bd48")
            w2cat_sb = cload(w2cat, [D1, D1], F32, "k_w2cat")
            b2rep_sb = cload(b2rep, [WINP, D1], F32, "k_b2rep")
            att2gb_sb = cload(att2gb, [WINP, 8 * D2], BF16, "k_att2gb")
            bias1rep_sb = cload(bias1rep, [WINP, D1], F32, "k_bias1rep")
            bias2rep_sb = cload(bias2rep, [WINP, D2], F32, "k_bias2rep")
            iota64_sb = cload(iota64, [WINP, G], F32, "k_iota64")
            ident_sb = cload(ident, [WINP, WINP], F32, "k_ident")

            idx16_sb = ipool.tile([WINP, NSUB * 8], I16)
            nc.sync.dma_start(out=idx16_sb[:], in_=idx16[:, :])
            cat2s_sb = ipool.tile([WINP, 2 * NSUB], BF16)
            nc.sync.dma_start(out=cat2s_sb[:], in_=cat2s[:, :])
            batchw_sb = ipool.tile([WINP, NW], F32)
            nc.sync.dma_start(out=batchw_sb[:], in_=batchw[:, :])

            # chunk layout for phase A: chunks of up to 512 edges
            chunks = []
            off = 0
            while off < EPW:
                cw = min(512, EPW - off)
                chunks.append((off, cw))
                off += cw

            # ------------------------------------------------ Phase A: layer 1
            with tc.tile_pool(name="ps_e", bufs=1, space="PSUM") as psE, \
                 tc.tile_pool(name="ps_epi", bufs=2, space="PSUM") as psEpi:
                for w in range(NW):
                    c16w = catpool.tile([7, EPW], BF16, tag="c16w")
                    nc.sync.dma_start(out=c16w[:], in_=cat16[w])
                    sww = catpool.tile([WINP, EPW], BF16, tag="sww")
                    nc.sync.dma_start(out=sww[:], in_=sw[w])
                    cat2v = cat2s_sb[:].rearrange("p (s d) -> p s d", d=2)

                    acc = psA.tile([WINP, 3 * H1], F32, tag="acc")
                    # rhs3[:, su, h*3+k]: k=0,1 -> ee*x_src[k]; k=2 -> ee
                    rhs3 = wpool.tile([WINP, T, 3 * H1], BF16, tag="rhsw")
                    for ci, (off, cw) in enumerate(chunks):
                        S = cw // 128
                        su0c = off // 128
                        u_ps = psB.tile([D1, 512], F32, tag="u")
                        nc.tensor.matmul(out=u_ps[:, :cw], lhsT=wcats_sb[:],
                                         rhs=c16w[:, off:off + cw],
                                         start=True, stop=True)
                        lrelu_u = wpool.tile([D1, 512], BF16, tag="lrelu")
                        nc.scalar.activation(out=lrelu_u[:, :cw], in_=u_ps[:, :cw],
                                             func=AT.Prelu, alpha=LRELU_ALPHA)
                        # e in slot-major directly: lhsT = lrelu_u subtile slice
                        e_ps = psE.tile([WINP, 4 * H1], F32, tag="eps")
                        for s in range(S):
                            nc.tensor.matmul(
                                out=e_ps[:, s * H1:(s + 1) * H1],
                                lhsT=lrelu_u[:, s * 128:(s + 1) * 128],
                                rhs=rsign_sb[:], start=True, stop=True)
                        nc.scalar.activation(
                            out=rhs3[:, su0c:su0c + S, :]
                                .rearrange("p s (h k) -> p s h k", k=3)[:, :, :, 2:3],
                            in_=e_ps[:, :S * H1]
                                .rearrange("p (s h) -> p s h", h=H1).unsqueeze(3),
                            func=AT.Exp)
                        # weighted 3-vector values + one-hot seg-sum
                        nc.vector.tensor_tensor(
                            out=rhs3[:, su0c:su0c + S, :]
                                .rearrange("p s (h k) -> p s h k", k=3)[:, :, :, 0:2],
                            in0=rhs3[:, su0c:su0c + S, :]
                                .rearrange("p s (h k) -> p s h k", k=3)[:, :, :, 2:3]
                                .broadcast_to([WINP, S, H1, 2]),
                            in1=cat2v[:, w * T + su0c:w * T + su0c + S, :]
                                .unsqueeze(2).broadcast_to([WINP, S, H1, 2]),
                            op=mybir.AluOpType.mult)
                        for s in range(S):
                            su = su0c + s
                            nc.tensor.matmul(out=acc[:],
                                             lhsT=sww[:, su * WINP:(su + 1) * WINP],
                                             rhs=rhs3[:, su, :],
                                             start=(su == 0),
                                             stop=(su == T - 1))

                    # window epilogue: s-hat = Num/Den, h = s-hat @ bd48 + bias1
                    den_sb = wpool.tile([WINP, H1], F32, tag="den")
                    nc.vector.tensor_scalar_max(
                        den_sb[:].unsqueeze(2),
                        acc[:].rearrange("p (h k) -> p h k", k=3)[:, :, 2:3], 1e-30)
                    rec_sb = wpool.tile([WINP, H1], F32, tag="rec")
                    nc.vector.reciprocal(rec_sb[:], den_sb[:])
                    s_sb = wpool.tile([WINP, 3 * H1], F32, tag="hsb")
                    nc.vector.tensor_tensor(
                        out=s_sb[:].rearrange("p (h k) -> p h k", k=3),
                        in0=acc[:].rearrange("p (h k) -> p h k", k=3),
                        in1=rec_sb[:].unsqueeze(2).broadcast_to([WINP, H1, 3]),
                        op=mybir.AluOpType.mult)
                    st_ps = psEpi.tile([D1, 512], F32, tag="epi")
                    nc.tensor.transpose(out=st_ps[:3 * H1, 0:WINP], in_=s_sb[:],
                                        identity=ident_sb[:])
                    st_sb = wpool.tile([3 * H1, WINP], F32, tag="stsb")
                    nc.vector.tensor_copy(st_sb[:], st_ps[:3 * H1, 0:WINP])
                    h_ps = psEpi.tile([WINP, 512], F32, tag="epi")
                    nc.tensor.matmul(out=h_ps[:, 0:D1], lhsT=st_sb[:],
                                     rhs=bd48_sb[:], start=True, stop=True)
                    hb_sb = wpool.tile([WINP, D1], F32, tag="hbsb")
                    nc.vector.tensor_add(hb_sb[:], h_ps[:, 0:D1], bias1rep_sb[:])
                    # ELU = relu(h) + exp(min(h,0)) - 1
                    neg_sb = wpool.tile([WINP, D1], F32, tag="negsb")
                    nc.vector.tensor_scalar_min(neg_sb[:], hb_sb[:], 0.0)
                    exp_sb = wpool.tile([WINP, D1], F32, tag="expsb")
                    nc.scalar.activation(out=exp_sb[:], in_=neg_sb[:], func=AT.Exp)
                    pos_sb = wpool.tile([WINP, D1], F32, tag="possb")
                    nc.vector.tensor_scalar_max(pos_sb[:], hb_sb[:], 0.0)
                    # helu holds ELU(h)+1; the -1 is folded into b2rep
                    helu = wpool.tile([WINP, D1], F32, tag="helu")
                    nc.vector.tensor_add(helu[:], pos_sb[:], exp_sb[:])
                    if DEBUG:
                        nc.sync.dma_start(out=dbg_h[w * WINP:(w + 1) * WINP, :],
                                          in_=helu[:])
                        nc.sync.dma_start(out=dbg_den[w * WINP:(w + 1) * WINP, :],
                                          in_=den_sb[:])

                    ht_ps = psEpi.tile([D1, 512], F32, tag="epi")
                    nc.tensor.transpose(out=ht_ps[:, 0:WINP], in_=helu[:],
                                        identity=ident_sb[:])
                    ht_sb = wpool.tile([D1, WINP], F32, tag="htsb")
                    nc.scalar.activation(out=ht_sb[:], in_=ht_ps[:, 0:WINP],
                                         func=AT.Copy)
                    x2_ps = psEpi.tile([WINP, 512], F32, tag="epi")
                    nc.tensor.matmul(out=x2_ps[:, 0:D1], lhsT=ht_sb[:],
                                     rhs=w2cat_sb[:], start=True, stop=True)
                    x2_sb = wpool.tile([WINP, D1], F32, tag="x2sb")
                    nc.vector.tensor_add(x2_sb[:], x2_ps[:, 0:D1], b2rep_sb[:])
                    xr2bf = wpool.tile([WINP, D2], BF16, tag="xr2bf")
                    nc.vector.tensor_copy(xr2bf[:], x2_sb[:, D2:D1])
                    rows = min(WN, NLOC - w * WN)
                    n0, n1 = w * WN, w * WN + rows
                    for k in range(NST):
                        a, b = max(n0, bounds[k]), min(n1, bounds[k + 1])
                        if a < b:
                            nc.sync.dma_start(
                                out=xl2locs[k][a - bounds[k]:b - bounds[k], :],
                                in_=x2_sb[a - n0:b - n0, 0:D2])
                    nc.sync.dma_start(out=xr2loc[w * WINP:w * WINP + WN, :],
                                      in_=xr2bf[:WN, :])
                    for k in range(NST):
                        if w == wcut[k] - 1:
                            nc.gpsimd.collective_compute(
                                "AllGather", mybir.AluOpType.bypass,
                                ins=[xl2locs[k][:, :]],
                                outs=[xl2buf[k][:, :]],
                                replica_groups=[list(range(NCORES))])

            # ------------------------------------------------ Phase B: layer 2
            with tc.tile_pool(name="ps_pool", bufs=1, space="PSUM") as psP, \
                 tc.tile_pool(name="gath", bufs=4) as gpool, \
                 tc.tile_pool(name="xgp", bufs=16) as xgpool:
                pool_ps = psP.tile([G, D2], F32)
                p0buf = ipool.tile([WINP, NW * (D2 + H2)], F32, tag="p0buf")

                def b_batch(w, su_off, so0, GB, acc2, swb, stw_sb, xr2w,
                            xg, start, stop):
                    # m2 = xr2[tgt] one-hot expansion + xe2 + xl2[src] (PSUM accum;
                    # the gathered xg rows enter via an identity matmul)
                    m2p = psB.tile([WINP, 512], F32, tag="u")
                    for j in range(GB):
                        so = so0 + j * 128
                        nc.tensor.matmul(out=m2p[:, j * D2:(j + 1) * D2],
                                         lhsT=stw_sb[:, so:so + 128],
                                         rhs=xr2w[:], start=(j == 0), stop=False)
                    nc.tensor.matmul(out=m2p[:, :GB * D2], lhsT=ident_sb[:],
                                     rhs=xg[:, :GB * D2], start=False, stop=True)
                    lr2 = wpool.tile([WINP, GB * D2], BF16, tag="lr2")
                    nc.scalar.activation(out=lr2[:], in_=m2p[:, :GB * D2],
                                         func=AT.Prelu, alpha=LRELU_ALPHA)
                    ta = wpool.tile([WINP, GB * D2], BF16, tag="ta")
                    nc.vector.tensor_tensor(
                        out=ta[:], in0=lr2[:], in1=att2gb_sb[:, :GB * D2],
                        op=mybir.AluOpType.mult)
                    e2 = wpool.tile([WINP, GB * H2], F32, tag="e2")
                    nc.vector.tensor_reduce(
                        out=e2[:],
                        in_=ta[:].rearrange("p (x c) -> p x c", c=C2),
                        axis=mybir.AxisListType.X, op=mybir.AluOpType.add)
                    rhs2 = wpool.tile([WINP, GB, D2 + H2], BF16, tag="rhs2")
                    nc.scalar.activation(
                        out=rhs2[:, :, D2:D2 + H2],
                        in_=e2[:].rearrange("p (g h) -> p g h", h=H2),
                        func=AT.Exp)
                    nc.vector.tensor_tensor(
                        out=rhs2[:, :, 0:D2].rearrange("p g (h c) -> p g h c", c=C2),
                        in0=xg[:, :GB * D2].rearrange("p (g h c) -> p g h c", h=H2, c=C2),
                        in1=rhs2[:, :, D2:D2 + H2].unsqueeze(3)
                            .broadcast_to([WINP, GB, H2, C2]),
                        op=mybir.AluOpType.mult)
                    for j in range(GB):
                        nc.tensor.matmul(
                            out=acc2[:],
                            lhsT=swb[:, (so0 + j * 128):(so0 + (j + 1) * 128)],
                            rhs=rhs2[:, j, :],
                            start=(start and j == 0),
                            stop=(stop and j == GB - 1))



                # ---- stage loops: stage k subtiles, one gather per window pair
                for k in range(NST):
                    last = (k == NST - 1)
                    GBk = Tk[k]
                    nidx_reg = nc.gpsimd.to_reg(GBk * WINP)
                    for w in range(NW):
                        xg2 = xgpool.tile([WINP, GBMAX * D2], F32, tag="xg")
                        base = (prefT[k] * NW + w * GBk) * 8
                        nc.gpsimd.dma_gather(
                            out_ap=xg2[:, :GBk * D2]
                                .rearrange("p (g d) -> p g d", d=D2),
                            in_ap=xl2buf[k][:, :],
                            idxs_ap=idx16_sb[:, base:base + GBk * 8],
                            num_idxs=GBk * WINP,
                            num_idxs_reg=nidx_reg,
                            elem_size=D2, queue_num=w % 4)
                        if True:
                            xr2w = gpool.tile([WINP, D2], BF16, tag="xr2w")
                            nc.sync.dma_start(out=xr2w[:],
                                              in_=xr2loc[w * WINP:(w + 1) * WINP, :])
                            c0 = prefT[k] * 128
                            c1 = (prefT[k] + GBk) * 128
                            stw_sb = catpool.tile([WINP, 10 * 128], BF16, tag="stw")
                            nc.scalar.dma_start(out=stw_sb[:, :c1 - c0],
                                                in_=stw[w, :, c0:c1])
                            swb = catpool.tile([WINP, 10 * 128], BF16, tag="swb")
                            nc.scalar.dma_start(out=swb[:, :c1 - c0],
                                                in_=sw[w, :, c0:c1])
                            acc2 = psA.tile([WINP, D2 + H2], F32, tag="acc")
                            b_batch(w, prefT[k], 0, GBk, acc2, swb, stw_sb, xr2w,
                                    xg2[:, 0:GBk * D2],
                                    True, True)
                        pslice = p0buf[:, w * (D2 + H2):(w + 1) * (D2 + H2)]
                        if k == 0:
                            nc.vector.tensor_copy(pslice, acc2[:])
                        elif not last:
                            nc.vector.tensor_add(pslice, pslice, acc2[:])
                        else:
                            comb = wpool.tile([WINP, D2 + H2], F32, tag="comb")
                            nc.vector.tensor_add(
                                comb[:], acc2[:],
                                p0buf[:, w * (D2 + H2):(w + 1) * (D2 + H2)])
                            den2 = wpool.tile([WINP, H2], F32, tag="den")
                            nc.vector.tensor_scalar_max(den2[:], comb[:, D2:D2 + H2], 1e-30)
                            rec2 = wpool.tile([WINP, H2], F32, tag="rec")
                            nc.vector.reciprocal(rec2[:], den2[:])
                            h2_sb = wpool.tile([WINP, D2], F32, tag="h2sb")
                            nc.vector.tensor_tensor(
                                out=h2_sb[:].rearrange("p (h c) -> p h c", c=C2),
                                in0=comb[:, 0:D2].rearrange("p (h c) -> p h c", c=C2),
                                in1=rec2[:].unsqueeze(2).broadcast_to([WINP, H2, C2]),
                                op=mybir.AluOpType.mult)
                            nc.vector.tensor_add(h2_sb[:], h2_sb[:], bias2rep_sb[:])
                            if DEBUG:
                                nc.sync.dma_start(out=dbg_h2[w * WINP:(w + 1) * WINP, :],
                                                  in_=h2_sb[:])
                            b_sb = wpool.tile([WINP, G], F32, tag="bsb")
                            nc.vector.tensor_tensor(
                                out=b_sb[:],
                                in0=batchw_sb[:, w:w + 1].to_broadcast([WINP, G]),
                                in1=iota64_sb[:],
                                op=mybir.AluOpType.is_equal)
                            nc.tensor.matmul(out=pool_ps[:], lhsT=b_sb[:], rhs=h2_sb[:],
                                             start=(w == 0), stop=(w == NW - 1))

                pool_sb = wpool.tile([G, D2], F32, tag="poolsb")
                nc.vector.tensor_copy(pool_sb[:], pool_ps[:])
                nc.sync.dma_start(out=pool_out[:, :], in_=pool_sb[:])

    nc.compile()
    return nc


_CACHE = {}


def kernel(**inputs):
    in_maps, T, counts = _prep(inputs)
    if T not in _CACHE:
        _CACHE[T] = _build(*T)
    nc = _CACHE[T]
    res = run_bass_kernel_spmd(nc, in_maps, core_ids=list(range(NCORES)))
    pool = np.zeros((G, D2), np.float64)
    for d in range(NCORES):
        pool += res.results[d]["pool_out"].astype(np.float64)
    out = pool / np.maximum(counts, 1.0)[:, None]
    return out.astype(np.float32)

